# revision 22
# baseline (speedup 1.0000x reference)
"""DistMult edge scoring on 8 Trainium2 NeuronCores.

score[e] = sum_d node_emb[src[e], d] * rel_emb[e, d] * node_emb[dst[e], d]

Strategy (data-parallel over edges, node table replicated per core):
  - Each of the 8 cores gets the full node_emb table in its HBM (fp16)
    plus a ~1/8 shard of the edges (fp16 rel rows + int16 src/dst
    indices). fp16 halves all HBM traffic; quantizing the three factors
    costs ~5e-4 L2 error on the scores (gate is 2e-2).
  - The node table is split into 4 blocks of 25000 rows; edges are
    bucketed by (src_block, dst_block) into 16 groups so both gathers
    address a <32768-row window (int16 local indices for the GPSIMD
    dma_gather ucode). Edges are dealt to cores per-group round-robin
    so every (core, group) count is ~4700 +-25, letting all groups
    share one CAP=5120 slot capacity (3.8 pct padding waste was 9.2).
  - Per group: one 5120-index head gather + one 5120-index tail gather
    + one strided rel load land [128, 40, 128] fp16 tiles with edge i
    at [i%128, i//128, :]. Gathers round-robin over 3 SWDGE queues
    (queue q runs its descriptor generation on GPSIMD core pair
    {2q,2q+1}, so queues parallelize desc-gen, the former bottleneck;
    queue 3 / nq=4 is pathologically slow - don't). num_idxs=5120 is
    an exact 5x64-descriptor packet multiple per SDMA engine; non-
    multiples (e.g. 4864) measure ~25 pct slower. DVE does two fp16
    multiplies + a blocked reduce over D=128 (f32 out) into a resident
    score plane, stored once at the end. bufs=6 on the tile pool (even
    rotation depths beat odd ones against the 3-queue rotation).
  - Per-rep HBM traffic/core: 2x21MB gathers + 21MB rel = 63MB, vs the
    358 GB/s per-core limit -> ~176us roofline; measured ~200us.
  - Host pre-permutes rel rows into the chunk layout and scatters the
    returned score planes back through the per-core edge id lists.
"""

import numpy as np

N_NODES = 100000
E_TOTAL = 600000
D = 128
N_CORES = 8
E_CORE = E_TOTAL // N_CORES  # 75000

NB = 4                # node blocks
BS = N_NODES // NB    # block size (rows per gather window)
G = NB * NB           # groups per core
CH = 5120             # edge slots per chunk
CAP = 5120            # slots per group
S = G * CAP           # total slots per core
COLS = S // 128       # score plane columns

_CACHE: dict = {}


def _build_module(repeats: int = 1):
    import concourse.bacc as bacc
    import concourse.mybir as mybir
    from concourse.tile import TileContext

    nc = bacc.Bacc(
        "TRN2",
        debug=False,
        enable_asserts=False,
        target_bir_lowering=False,
        num_devices=N_CORES,
        num_swdge_queues=3,
    )
    f32 = mybir.dt.float32
    f16 = mybir.dt.float16
    i16 = mybir.dt.int16

    node = nc.dram_tensor("node_emb", [N_NODES, D], f16, kind="ExternalInput").ap()
    relsw = nc.dram_tensor("relsw", [128, S], f16, kind="ExternalInput").ap()
    srci = nc.dram_tensor("srci", [128, S // 16], i16, kind="ExternalInput").ap()
    dsti = nc.dram_tensor("dsti", [128, S // 16], i16, kind="ExternalInput").ap()
    out = nc.dram_tensor("scores", [128, COLS], f32, kind="ExternalOutput").ap()

    n_chunks = CAP // CH

    with TileContext(nc) as tc:
        with (
            tc.tile_pool(name="idx", bufs=1) as idxp,
            tc.tile_pool(name="hd", bufs=3) as hdp,
            tc.tile_pool(name="big", bufs=5) as bigp,
            tc.tile_pool(name="res", bufs=1) as resp,
        ):
            src_t = idxp.tile([128, S // 16], i16, tag="srci")
            dst_t = idxp.tile([128, S // 16], i16, tag="dsti")
            score_t = resp.tile([128, COLS], f32, tag="score")
            nc.sync.dma_start(out=src_t[:], in_=srci[:])
            nc.sync.dma_start(out=dst_t[:], in_=dsti[:])

            qi = 0
            for _rep in range(repeats):
              for g in range(G):
                sb = (g // NB) * BS
                db = (g % NB) * BS
                s0 = g * CAP
                if g % 2 == 0:
                    # one head gather covers groups g, g+1 (same src block:
                    # slot order is src-block-major with 4 groups per block)
                    head2 = hdp.tile([128, 2 * CAP], f16, tag="head")
                    nc.gpsimd.dma_gather(
                        out_ap=head2[:].rearrange("p (c d) -> p c d", d=D),
                        in_ap=node[sb : sb + BS],
                        idxs_ap=src_t[:, s0 // 16 : (s0 + 2 * CAP) // 16],
                        num_idxs=2 * CAP,
                        num_idxs_reg=2 * CAP,
                        elem_size=D,
                        single_packet=False,
                        queue_num=qi % 3,
                    )
                    qi += 1
                hcols = slice((g % 2) * CAP, (g % 2 + 1) * CAP)
                tail = bigp.tile([128, CAP], f16, tag="tail")
                relt = bigp.tile([128, CAP], f16, tag="rel")
                nc.gpsimd.dma_gather(
                    out_ap=tail[:].rearrange("p (c d) -> p c d", d=D),
                    in_ap=node[db : db + BS],
                    idxs_ap=dst_t[:, s0 // 16 : (s0 + CAP) // 16],
                    num_idxs=CAP,
                    num_idxs_reg=CAP,
                    elem_size=D,
                    single_packet=False,
                    queue_num=qi % 3,
                )
                qi += 1
                nc.sync.dma_start(out=relt[:], in_=relsw[:, s0 : s0 + CAP])
                nc.vector.tensor_tensor(
                    out=relt[:], in0=relt[:], in1=head2[:, hcols],
                    op=mybir.AluOpType.mult,
                )
                nc.vector.tensor_tensor(
                    out=relt[:], in0=relt[:], in1=tail[:],
                    op=mybir.AluOpType.mult,
                )
                nc.vector.tensor_reduce(
                    out=score_t[:, s0 // 128 : (s0 + CAP) // 128],
                    in_=relt[:].rearrange("p (c d) -> p c d", d=D),
                    axis=mybir.AxisListType.X,
                    op=mybir.AluOpType.add,
                )

            nc.sync.dma_start(out=out[:], in_=score_t[:])

    nc.compile()
    return nc


def _get_module(repeats: int = 1):
    key = ("nc", repeats)
    if key not in _CACHE:
        _CACHE[key] = _build_module(repeats)
    return _CACHE[key]


def _wrap16(x: np.ndarray) -> np.ndarray:
    """[S] int16 -> [128, S/16] gather index plane (16-wrap, replicated 8x)."""
    w = x.reshape(S // 16, 16).T
    return np.ascontiguousarray(np.tile(w, (8, 1)))


def _prep_core(rel_c, src_c, dst_c):
    """rel/src/dst are this core's edges, already sorted by group id."""
    src_c = src_c.astype(np.int64)
    dst_c = dst_c.astype(np.int64)
    n = len(src_c)
    gs = (src_c // BS) * NB + (dst_c // BS)
    counts = np.bincount(gs, minlength=G)
    if counts.max() > CAP:
        raise ValueError(f"group overflow: {counts.max()} > CAP={CAP}")
    cum = np.zeros(G, dtype=np.int64)
    cum[1:] = np.cumsum(counts)[:-1]
    rank = np.arange(n) - cum[gs]
    slots = gs * CAP + rank  # slot for each edge

    loc_src = np.zeros(S, dtype=np.int16)
    loc_dst = np.zeros(S, dtype=np.int16)
    loc_src[slots] = (src_c - (gs // NB) * BS).astype(np.int16)
    loc_dst[slots] = (dst_c - (gs % NB) * BS).astype(np.int16)

    rel_perm = np.zeros((S, D), dtype=np.float16)
    rel_perm[slots] = rel_c.astype(np.float16)
    relsw = np.ascontiguousarray(
        rel_perm.reshape(S // 128, 128, D).transpose(1, 0, 2).reshape(128, S)
    )
    return (
        {"relsw": relsw, "srci": _wrap16(loc_src), "dsti": _wrap16(loc_dst)},
        slots,
    )


def make_in_maps(node_emb, rel_emb, src, dst):
    node = np.ascontiguousarray(np.asarray(node_emb, dtype=np.float16))
    rel_emb = np.asarray(rel_emb, dtype=np.float32)
    src = np.asarray(src)
    dst = np.asarray(dst)

    # Balance (core, group) counts: sort all edges by group id, then deal
    # each group's edges in 8 near-equal contiguous chunks, chunk c ->
    # core c. Max per-core group count drops from mean+3.4sigma (~4892)
    # to ~ceil(total_g/8) (~4720), allowing the smaller CAP.
    ge = (src // BS) * NB + (dst // BS)
    order_g = np.argsort(ge, kind="stable")
    ge_sorted = ge[order_g]
    totals = np.bincount(ge_sorted, minlength=G)
    starts = np.zeros(G + 1, dtype=np.int64)
    starts[1:] = np.cumsum(totals)
    core_edge_ids = [[] for _ in range(N_CORES)]
    for g in range(G):
        grp = order_g[starts[g] : starts[g + 1]]
        bounds = np.linspace(0, len(grp), N_CORES + 1).astype(np.int64)
        for c in range(N_CORES):
            core_edge_ids[c].append(grp[bounds[c] : bounds[c + 1]])

    in_maps, metas = [], []
    for c in range(N_CORES):
        ids = np.concatenate(core_edge_ids[c])  # group-sorted by construction
        m, slots = _prep_core(rel_emb[ids], src[ids], dst[ids])
        m["node_emb"] = node
        in_maps.append(m)
        metas.append((ids, slots))
    return in_maps, metas


def gather_outputs(results, metas) -> np.ndarray:
    scores = np.empty(E_TOTAL, dtype=np.float32)
    for c in range(N_CORES):
        plane = np.asarray(results[c]["scores"], dtype=np.float32)  # [128, COLS]
        lin = plane.T.ravel()  # lin[slot], slot = col*128 + p
        ids, slots = metas[c]
        scores[ids] = lin[slots]
    return scores


def run(node_emb, rel_emb, src, dst, trace=False):
    from concourse import bass_utils
    from concourse.bass_interp import get_hw_module

    nc = _get_module()
    in_maps, metas = make_in_maps(node_emb, rel_emb, src, dst)
    old_m = nc.m
    nc.m = get_hw_module(nc.m)
    try:
        res = bass_utils.run_bass_kernel_spmd(
            nc, in_maps, core_ids=list(range(N_CORES)), trace=trace
        )
    finally:
        nc.m = old_m
    return gather_outputs(res.results, metas), res


def kernel(node_emb, rel_emb, src, dst):
    scores, _ = run(node_emb, rel_emb, src, dst, trace=False)
    return scores



# revision 23
# speedup vs baseline: 1.9264x; 1.9264x over previous
"""DistMult edge scoring on 8 Trainium2 NeuronCores.

score[e] = sum_d node_emb[src[e], d] * rel_emb[e, d] * node_emb[dst[e], d]

Strategy (data-parallel over edges, node table replicated per core):
  - Each of the 8 cores gets the full node_emb table in its HBM (fp16)
    plus a ~1/8 shard of the edges (fp16 rel rows + int16 src/dst
    indices). fp16 halves all HBM traffic; quantizing the three factors
    costs ~5e-4 L2 error on the scores (gate is 2e-2).
  - The node table is split into 4 blocks of 25000 rows; edges are
    bucketed by (src_block, dst_block) into 16 groups so both gathers
    address a <32768-row window (int16 local indices for the GPSIMD
    dma_gather ucode). Edges are dealt to cores per-group round-robin
    so every (core, group) count is ~4700 +-25, letting all groups
    share one CAP=5120 slot capacity (3.8 pct padding waste was 9.2).
  - Per group: one 5120-index head gather + one 5120-index tail gather
    + one strided rel load land [128, 40, 128] fp16 tiles with edge i
    at [i%128, i//128, :]. Gathers round-robin over 3 SWDGE queues
    (queue q runs its descriptor generation on GPSIMD core pair
    {2q,2q+1}, so queues parallelize desc-gen, the former bottleneck;
    queue 3 / nq=4 is pathologically slow - don't). num_idxs=5120 is
    an exact 5x64-descriptor packet multiple per SDMA engine; non-
    multiples (e.g. 4864) measure ~25 pct slower. DVE does two fp16
    multiplies + a blocked reduce over D=128 (f32 out) into a resident
    score plane, stored once at the end. bufs=6 on the tile pool (even
    rotation depths beat odd ones against the 3-queue rotation).
  - Per-rep HBM traffic/core: 2x21MB gathers + 21MB rel = 63MB, vs the
    358 GB/s per-core limit -> ~176us roofline; measured ~200us.
  - Host pre-permutes rel rows into the chunk layout and scatters the
    returned score planes back through the per-core edge id lists.
"""

import numpy as np

N_NODES = 100000
E_TOTAL = 600000
D = 128
N_CORES = 8
E_CORE = E_TOTAL // N_CORES  # 75000

NB = 4                # node blocks
BS = N_NODES // NB    # block size (rows per gather window)
G = NB * NB           # groups per core
CH = 5120             # edge slots per chunk
CAP = 5120            # slots per group
S = G * CAP           # total slots per core
COLS = S // 128       # score plane columns

_CACHE: dict = {}


def _build_module(repeats: int = 1):
    import concourse.bacc as bacc
    import concourse.mybir as mybir
    from concourse.tile import TileContext

    nc = bacc.Bacc(
        "TRN2",
        debug=False,
        enable_asserts=False,
        target_bir_lowering=False,
        num_devices=N_CORES,
        num_swdge_queues=3,
    )
    f32 = mybir.dt.float32
    f16 = mybir.dt.float16
    i16 = mybir.dt.int16

    node = nc.dram_tensor("node_emb", [N_NODES, D], f16, kind="ExternalInput").ap()
    relsw = nc.dram_tensor("relsw", [128, S], f16, kind="ExternalInput").ap()
    srci = nc.dram_tensor("srci", [128, S // 16], i16, kind="ExternalInput").ap()
    dsti = nc.dram_tensor("dsti", [128, S // 16], i16, kind="ExternalInput").ap()
    out = nc.dram_tensor("scores", [128, COLS], f32, kind="ExternalOutput").ap()

    n_chunks = CAP // CH

    with TileContext(nc) as tc:
        with (
            tc.tile_pool(name="idx", bufs=1) as idxp,
            tc.tile_pool(name="big", bufs=6) as bigp,
            tc.tile_pool(name="res", bufs=1) as resp,
        ):
            src_t = idxp.tile([128, S // 16], i16, tag="srci")
            dst_t = idxp.tile([128, S // 16], i16, tag="dsti")
            score_t = resp.tile([128, COLS], f32, tag="score")
            nc.sync.dma_start(out=src_t[:], in_=srci[:])
            nc.sync.dma_start(out=dst_t[:], in_=dsti[:])

            qi = 0
            for _rep in range(repeats):
              for g in range(G):
                sb = (g // NB) * BS
                db = (g % NB) * BS
                for c in range(n_chunks):
                    s0 = g * CAP + c * CH
                    head = bigp.tile([128, CH], f16, tag="head")
                    tail = bigp.tile([128, CH], f16, tag="tail")
                    relt = bigp.tile([128, CH], f16, tag="rel")
                    nc.gpsimd.dma_gather(
                        out_ap=head[:].rearrange("p (c d) -> p c d", d=D),
                        in_ap=node[sb : sb + BS],
                        idxs_ap=src_t[:, s0 // 16 : (s0 + CH) // 16],
                        num_idxs=CH,
                        num_idxs_reg=CH,
                        elem_size=D,
                        single_packet=False,
                        queue_num=qi % 3,
                    )
                    qi += 1
                    nc.gpsimd.dma_gather(
                        out_ap=tail[:].rearrange("p (c d) -> p c d", d=D),
                        in_ap=node[db : db + BS],
                        idxs_ap=dst_t[:, s0 // 16 : (s0 + CH) // 16],
                        num_idxs=CH,
                        num_idxs_reg=CH,
                        elem_size=D,
                        single_packet=False,
                        queue_num=qi % 3,
                    )
                    qi += 1
                    nc.sync.dma_start(out=relt[:], in_=relsw[:, s0 : s0 + CH])
                    nc.vector.tensor_tensor(
                        out=head[:], in0=head[:], in1=relt[:],
                        op=mybir.AluOpType.mult,
                    )
                    nc.vector.tensor_tensor(
                        out=head[:], in0=head[:], in1=tail[:],
                        op=mybir.AluOpType.mult,
                    )
                    nc.vector.tensor_reduce(
                        out=score_t[:, s0 // 128 : (s0 + CH) // 128],
                        in_=head[:].rearrange("p (c d) -> p c d", d=D),
                        axis=mybir.AxisListType.X,
                        op=mybir.AluOpType.add,
                    )

            nc.sync.dma_start(out=out[:], in_=score_t[:])

    nc.compile()
    return nc


def _get_module(repeats: int = 1):
    key = ("nc", repeats)
    if key not in _CACHE:
        _CACHE[key] = _build_module(repeats)
    return _CACHE[key]


def _wrap16(x: np.ndarray) -> np.ndarray:
    """[S] int16 -> [128, S/16] gather index plane (16-wrap, replicated 8x)."""
    w = x.reshape(S // 16, 16).T
    return np.ascontiguousarray(np.tile(w, (8, 1)))


def _prep_core(rel_c, src_c, dst_c):
    """rel/src/dst are this core's edges, already sorted by group id."""
    src_c = src_c.astype(np.int64)
    dst_c = dst_c.astype(np.int64)
    n = len(src_c)
    gs = (src_c // BS) * NB + (dst_c // BS)
    counts = np.bincount(gs, minlength=G)
    if counts.max() > CAP:
        raise ValueError(f"group overflow: {counts.max()} > CAP={CAP}")
    cum = np.zeros(G, dtype=np.int64)
    cum[1:] = np.cumsum(counts)[:-1]
    rank = np.arange(n) - cum[gs]
    slots = gs * CAP + rank  # slot for each edge

    loc_src = np.zeros(S, dtype=np.int16)
    loc_dst = np.zeros(S, dtype=np.int16)
    loc_src[slots] = (src_c - (gs // NB) * BS).astype(np.int16)
    loc_dst[slots] = (dst_c - (gs % NB) * BS).astype(np.int16)

    rel_perm = np.zeros((S, D), dtype=np.float16)
    rel_perm[slots] = rel_c.astype(np.float16)
    relsw = np.ascontiguousarray(
        rel_perm.reshape(S // 128, 128, D).transpose(1, 0, 2).reshape(128, S)
    )
    return (
        {"relsw": relsw, "srci": _wrap16(loc_src), "dsti": _wrap16(loc_dst)},
        slots,
    )


def make_in_maps(node_emb, rel_emb, src, dst):
    node = np.ascontiguousarray(np.asarray(node_emb, dtype=np.float16))
    rel_emb = np.asarray(rel_emb, dtype=np.float32)
    src = np.asarray(src)
    dst = np.asarray(dst)

    # Balance (core, group) counts: sort all edges by group id, then deal
    # each group's edges in 8 near-equal contiguous chunks, chunk c ->
    # core c. Max per-core group count drops from mean+3.4sigma (~4892)
    # to ~ceil(total_g/8) (~4720), allowing the smaller CAP.
    ge = (src // BS) * NB + (dst // BS)
    order_g = np.argsort(ge, kind="stable")
    ge_sorted = ge[order_g]
    totals = np.bincount(ge_sorted, minlength=G)
    starts = np.zeros(G + 1, dtype=np.int64)
    starts[1:] = np.cumsum(totals)
    core_edge_ids = [[] for _ in range(N_CORES)]
    for g in range(G):
        grp = order_g[starts[g] : starts[g + 1]]
        bounds = np.linspace(0, len(grp), N_CORES + 1).astype(np.int64)
        for c in range(N_CORES):
            core_edge_ids[c].append(grp[bounds[c] : bounds[c + 1]])

    in_maps, metas = [], []
    for c in range(N_CORES):
        ids = np.concatenate(core_edge_ids[c])  # group-sorted by construction
        m, slots = _prep_core(rel_emb[ids], src[ids], dst[ids])
        m["node_emb"] = node
        in_maps.append(m)
        metas.append((ids, slots))
    return in_maps, metas


def gather_outputs(results, metas) -> np.ndarray:
    scores = np.empty(E_TOTAL, dtype=np.float32)
    for c in range(N_CORES):
        plane = np.asarray(results[c]["scores"], dtype=np.float32)  # [128, COLS]
        lin = plane.T.ravel()  # lin[slot], slot = col*128 + p
        ids, slots = metas[c]
        scores[ids] = lin[slots]
    return scores


def run(node_emb, rel_emb, src, dst, trace=False):
    from concourse import bass_utils
    from concourse.bass_interp import get_hw_module

    nc = _get_module()
    in_maps, metas = make_in_maps(node_emb, rel_emb, src, dst)
    old_m = nc.m
    nc.m = get_hw_module(nc.m)
    try:
        res = bass_utils.run_bass_kernel_spmd(
            nc, in_maps, core_ids=list(range(N_CORES)), trace=trace
        )
    finally:
        nc.m = old_m
    return gather_outputs(res.results, metas), res


def kernel(node_emb, rel_emb, src, dst):
    scores, _ = run(node_emb, rel_emb, src, dst, trace=False)
    return scores

